# revision 8
# baseline (speedup 1.0000x reference)
"""Trainium2 Bass kernel for nn_ConvAttention_34600256537137.

Math notes (validated against the reference to ~3e-6 rel err):
  qkv = 1x1conv(x, w1)+b1 -> Q,K,V;  score = conv5x5(Q_s)+conv5x5(K_t)+b2;
  attn = softmax_t(score);  out = einsum(attn, V).
  Softmax over t is shift-invariant, so the Q-half of the score (constant in t),
  b2, and the bias contributions all cancel.  The computation collapses to:
    weff[ci,dy,dx] = sum_c w1K[c,ci] * w2K[c,dy,dx]        (host, tiny)
    sK[b,t,h,w]    = conv5x5_reflect(x[b,:,:,:,t], weff)
    attn[b,t,h,w]  = softmax_t(sK)
    out_pre[b,o,h,w] = sum_{ci,t} w1V[o,ci] * attn[b,t,h,w] * x[b,ci,h,w,t] + b1V[o]
    out[b,o,h,w,s] = out_pre[b,o,h,w]   (s-independent)

Sharding: 8 cores = (b in {0,1}) x (4 chunks of 8 rows of H).  All reflect
padding and layout transforms are precomputed host-side so every core runs an
identical program on its own slices.

Device program per core:
  - score conv as 60 PSUM-accumulating matmuls: stationary = banded weight
    matrices S[(r,dx)][ci, m] (host-built), moving = slab row windows.
    psum_sk[8, 32, 16] = sK for this core's 8 output rows.
  - softmax over t (no max subtraction needed: |score| <~ 6).
  - attn broadcast to [(ci8,t), (h,w)] via small SBUF->SBUF DMAs.
  - xattn = x_t * attn_b  (DVE), then 8 accumulating matmuls with
    K=(ci8,t)=128 contract channels AND the t-reduction in one pass.
  - bias-add via ScalarE during PSUM->SBUF copy; s-broadcast on the store DMA.
"""

import sys

if "/opt/trn_rl_repo" not in sys.path:
    sys.path.insert(0, "/opt/trn_rl_repo")

import numpy as np

B, C, H, W, S = 2, 64, 32, 32, 16
KS, PAD = 5, 2
NCORES = 8
ROWS = H // 4          # output rows per core
SLAB_R = ROWS + 2 * PAD  # 12
SLAB_W = W + 2 * PAD     # 36
NMM = SLAB_R * KS        # 60 score matmuls

_MODULE = None


def _build_module():
    import concourse.bacc as bacc
    import concourse.bass as bass
    import concourse.tile as tile
    from concourse import mybir

    f32 = mybir.dt.float32
    nc = bacc.Bacc("TRN2", target_bir_lowering=False, debug=False, num_devices=NCORES)

    slab_d = nc.dram_tensor("slab", [C, SLAB_R, SLAB_W, S], f32, kind="ExternalInput")
    xt_d = nc.dram_tensor("xt", [128, 8, ROWS * W], f32, kind="ExternalInput")
    sall_d = nc.dram_tensor("sall", [C, NMM, ROWS], f32, kind="ExternalInput")
    w1vr_d = nc.dram_tensor("w1vr", [128, 8, C], f32, kind="ExternalInput")
    b1v_d = nc.dram_tensor("b1v", [C, 1], f32, kind="ExternalInput")
    o_d = nc.dram_tensor("o", [C, S, ROWS * W], f32, kind="ExternalOutput")

    with tile.TileContext(nc) as tc:
        with tc.tile_pool(name="sb", bufs=1) as sb, tc.tile_pool(
            name="ps", bufs=1, space="PSUM"
        ) as ps:
            s_slab = sb.tile([C, SLAB_R, SLAB_W, S], f32)
            nc.sync.dma_start(s_slab, slab_d.ap())
            s_sall = sb.tile([C, NMM, ROWS], f32)
            nc.sync.dma_start(s_sall, sall_d.ap())
            s_xt = sb.tile([128, 8, ROWS * W], f32)
            nc.sync.dma_start(s_xt, xt_d.ap())
            s_w1vr = sb.tile([128, 8, C], f32)
            nc.sync.dma_start(s_w1vr, w1vr_d.ap())
            s_b1v = sb.tile([C, 1], f32)
            nc.sync.dma_start(s_b1v, b1v_d.ap())

            # --- score conv: sK[m, w, t] via 60 accumulating matmuls ---
            p_sk = ps.tile([ROWS, W, S], f32)
            for i in range(NMM):
                r, dxi = divmod(i, KS)
                nc.tensor.matmul(
                    p_sk,
                    s_sall[:, i, :],                      # stationary [C, 8]
                    s_slab[:, r, dxi : dxi + W, :],       # moving [C, 32, 16]
                    start=(i == 0),
                    stop=(i == NMM - 1),
                )

            # --- softmax over t (last axis); scores are small, skip max-sub ---
            s_exp = sb.tile([ROWS, W, S], f32)
            nc.scalar.activation(s_exp, p_sk, mybir.ActivationFunctionType.Exp)
            s_den = sb.tile([ROWS, W], f32)
            nc.vector.tensor_reduce(
                s_den, s_exp, axis=mybir.AxisListType.X, op=mybir.AluOpType.add
            )
            s_rcp = sb.tile([ROWS, W], f32)
            nc.vector.reciprocal(s_rcp, s_den)
            # attn stored w-contiguous ([h, t, w]) so the DRAM bounce below
            # keeps every DMA's fastest-moving dim contiguous.
            s_attn_T = sb.tile([ROWS, S, W], f32)
            nc.vector.tensor_tensor(
                s_attn_T.transpose([0, 2, 1]),      # view as [h, w, t]
                s_exp,
                s_rcp.unsqueeze(2).broadcast_to((ROWS, W, S)),
                op=mybir.AluOpType.mult,
            )

            # --- broadcast attn to [(ci8,t), (h,w)] via a DRAM bounce ---
            # (SBUF APs can't express partition-crossing broadcasts; DRAM is flat)
            d_attn = nc.dram_tensor("d_attn", [S, ROWS, W], f32)
            nc.sync.dma_start(
                d_attn.ap().transpose([1, 0, 2]),   # iterate (h, t, w)
                s_attn_T,
            )
            s_attnb = sb.tile([128, ROWS * W], f32)
            for g in range(8):
                src = bass.AP(
                    tensor=d_attn.ap().tensor,
                    offset=0,
                    ap=[[ROWS * W, S], [1, ROWS * W]],  # (t, hw)
                )
                nc.sync.dma_start(s_attnb[g * S : (g + 1) * S, :], src)

            # --- xattn = x_t * attn ; contract (ci,t) on PE ---
            s_xa = sb.tile([128, 8, ROWS * W], f32)
            nc.vector.tensor_tensor(
                s_xa,
                s_xt,
                s_attnb.unsqueeze(1).broadcast_to((128, 8, ROWS * W)),
                op=mybir.AluOpType.mult,
            )
            p_o = ps.tile([C, ROWS * W], f32)
            for g in range(8):
                nc.tensor.matmul(
                    p_o,
                    s_w1vr[:, g, :],
                    s_xa[:, g, :],
                    start=(g == 0),
                    stop=(g == 7),
                )

            # --- bias add + copy to SBUF, then s-broadcast store ---
            s_o = sb.tile([C, ROWS * W], f32)
            nc.scalar.activation(
                s_o,
                p_o,
                mybir.ActivationFunctionType.Identity,
                bias=s_b1v[:, 0:1],
                scale=1.0,
            )
            nc.sync.dma_start(
                o_d.ap(), s_o.unsqueeze(1).broadcast_to((C, S, ROWS * W))
            )

    nc.compile()
    return nc


def _get_module():
    global _MODULE
    if _MODULE is None:
        _MODULE = _build_module()
    return _MODULE


def make_host_inputs(x, w1, b1, w2, b2):
    """Host-side precompute: folded weights + per-core reflect-padded slices."""
    x = np.ascontiguousarray(np.asarray(x, np.float32))
    w1 = np.asarray(w1, np.float32)
    b1 = np.asarray(b1, np.float32)
    w2 = np.asarray(w2, np.float32)

    w1K = w1[C : 2 * C, :, 0, 0]          # [c, ci]
    w2K = w2[0, C : 2 * C]                # [c, 5, 5]
    weff = np.einsum("ci,cyx->iyx", w1K, w2K)  # [ci, dyi, dxi]
    w1V = w1[2 * C :, :, 0, 0]            # [co, ci]
    b1v = np.ascontiguousarray(b1[2 * C :].reshape(C, 1))

    sall = np.zeros((C, NMM, ROWS), np.float32)
    for r in range(SLAB_R):
        for dxi in range(KS):
            i = r * KS + dxi
            for m in range(ROWS):
                dyi = r - m
                if 0 <= dyi < KS:
                    sall[:, i, m] = weff[:, dyi, dxi]

    # w1vr[(ci8,t), g, co] = w1V[co, 8g+ci8]
    tmp = w1V.T.reshape(8, 8, C)                      # (g, ci8, co)
    w1vr = np.ascontiguousarray(
        np.broadcast_to(tmp[:, :, None, :], (8, 8, S, C))
        .transpose(1, 2, 0, 3)
        .reshape(128, 8, C)
    )

    in_maps = []
    for core in range(NCORES):
        b, hc = divmod(core, 4)
        h0 = ROWS * hc
        xp = np.pad(x[b], ((0, 0), (PAD, PAD), (PAD, PAD), (0, 0)), mode="reflect")
        slab = np.ascontiguousarray(xp[:, h0 : h0 + SLAB_R, :, :])
        xs = x[b][:, h0 : h0 + ROWS, :, :]            # [ci, h, w, t]
        xt = np.ascontiguousarray(
            xs.reshape(8, 8, ROWS, W, S)
            .transpose(1, 4, 0, 2, 3)
            .reshape(128, 8, ROWS * W)
        )
        in_maps.append(
            {"slab": slab, "xt": xt, "sall": sall, "w1vr": w1vr, "b1v": b1v}
        )
    return in_maps


def assemble_output(results):
    out = np.zeros((B, C, H, W, S), np.float32)
    for core in range(NCORES):
        b, hc = divmod(core, 4)
        h0 = ROWS * hc
        o = results[core]["o"].reshape(C, S, ROWS, W).transpose(0, 2, 3, 1)
        out[b, :, h0 : h0 + ROWS, :, :] = o
    return out


def kernel(x, w1, b1, w2, b2):
    from concourse.bass_utils import run_bass_kernel_spmd

    nc = _get_module()
    in_maps = make_host_inputs(x, w1, b1, w2, b2)
    res = run_bass_kernel_spmd(nc, in_maps, core_ids=list(range(NCORES)))
    return assemble_output(res.results)


# revision 9
# speedup vs baseline: 1.4717x; 1.4717x over previous
"""Trainium2 Bass kernel for nn_ConvAttention_34600256537137.

Math notes (validated against the reference to ~3e-6 rel err):
  qkv = 1x1conv(x, w1)+b1 -> Q,K,V;  score = conv5x5(Q_s)+conv5x5(K_t)+b2;
  attn = softmax_t(score);  out = einsum(attn, V).
  Softmax over t is shift-invariant, so the Q-half of the score (constant in
  t), b2, and the K-path bias all cancel.  The computation collapses to:
    weff[ci,dy,dx] = sum_c w1K[c,ci] * w2K[c,dy,dx]        (host, tiny)
    sK[b,t,h,w]    = conv5x5_reflect(x[b,:,:,:,t], weff)
    attn[b,t,h,w]  = softmax_t(sK)
    out_pre[b,o,h,w] = sum_{ci,t} w1V[o,ci] * attn[b,t,h,w] * x[b,ci,h,w,t]
    out[b,o,h,w,s] = out_pre[b,o,h,w] + b1V[o]   (s-independent; bias on host)

Sharding: 8 cores = (b in {0,1}) x (4 chunks of 8 rows of H).  All reflect
padding and layout transforms are precomputed host-side so every core runs an
identical program on its own slices.

Device program per core:
  - score conv phase 1 on PE: T[tap, pos] = weff^T @ slab, streamed once
    (24 half-row matmuls into [25, 288] PSUM chunks).
  - PSUM->SBUF copies write T transposed to (tap, t, row, w) so every
    subsequent DMA keeps its fastest-moving dim contiguous.
  - DRAM bounce + 25 per-tap gather DMAs build R[(t,h), tap, w]; DVE reduces
    over taps at full 128-lane occupancy.
  - softmax over t via one more 16KB bounce into [h, (t, w)] layout.
  - V path: xattn = x_t * attn_b (DVE), 8 accumulating matmuls with
    K=(ci8,t)=128 contract channels AND the t-reduction in one pass.
"""

import sys

if "/opt/trn_rl_repo" not in sys.path:
    sys.path.insert(0, "/opt/trn_rl_repo")

import numpy as np

B, C, H, W, S = 2, 64, 32, 32, 16
KS, PAD = 5, 2
NCORES = 8
ROWS = H // 4            # output rows per core
SLAB_R = ROWS + 2 * PAD  # 12
SLAB_W = W + 2 * PAD     # 36
NTAP = KS * KS           # 25
NPOS = SLAB_R * SLAB_W * S  # 6912 slab positions
HW = ROWS * W            # 256 output positions

_MODULE = None


def _build_module():
    import concourse.bacc as bacc
    import concourse.bass as bass
    import concourse.tile as tile
    from concourse import mybir

    f32 = mybir.dt.float32
    AF = mybir.ActivationFunctionType
    ALU = mybir.AluOpType
    nc = bacc.Bacc("TRN2", target_bir_lowering=False, debug=False, num_devices=NCORES)

    slab_d = nc.dram_tensor("slab", [C, SLAB_R, SLAB_W, S], f32, kind="ExternalInput")
    xt_d = nc.dram_tensor("xt", [128, 8, HW], f32, kind="ExternalInput")
    weff_d = nc.dram_tensor("weff", [C, NTAP], f32, kind="ExternalInput")
    w1vr_d = nc.dram_tensor("w1vr", [128, 8, C], f32, kind="ExternalInput")
    o_d = nc.dram_tensor("o", [C, S, HW], f32, kind="ExternalOutput")

    # scratch DRAM for partition-crossing rearrangements
    td_d = nc.dram_tensor("td", [NTAP, S, SLAB_R, SLAB_W], f32)   # T, t-major
    skd_d = nc.dram_tensor("skd", [S, ROWS, W], f32)              # sK, t-major
    attnd_d = nc.dram_tensor("attnd", [S, ROWS, W], f32)          # attn, t-major

    with tile.TileContext(nc) as tc:
        with tc.tile_pool(name="sb", bufs=1) as sb, tc.tile_pool(
            name="ps", bufs=4, space="PSUM"
        ) as ps, tc.tile_pool(name="pso", bufs=1, space="PSUM") as pso:
            s_slab = sb.tile([C, SLAB_R, SLAB_W, S], f32)
            nc.sync.dma_start(s_slab, slab_d.ap())
            s_weff = sb.tile([C, NTAP], f32)
            nc.sync.dma_start(s_weff, weff_d.ap())
            s_xt = sb.tile([128, 8, HW], f32)
            nc.sync.dma_start(s_xt, xt_d.ap())
            s_w1vr = sb.tile([128, 8, C], f32)
            nc.sync.dma_start(s_w1vr, w1vr_d.ap())

            # --- phase 1: T[tap, (row, w, t)] = weff^T @ slab, half-row chunks
            # s_T2 holds T transposed to (tap, t, row, w): w contiguous.
            s_T2 = sb.tile([NTAP, S, SLAB_R, SLAB_W], f32)
            HREST = SLAB_W // 2  # 18
            for hr in range(SLAB_R * 2):
                row, half = divmod(hr, 2)
                p_t = ps.tile([NTAP, HREST, S], f32, tag="pt")
                nc.tensor.matmul(
                    p_t,
                    s_weff,
                    s_slab[:, row, half * HREST : (half + 1) * HREST, :],
                    start=True,
                    stop=True,
                )
                # copy PSUM -> s_T2[(tap), t, row, w-half] (strided write)
                eng = nc.vector if hr % 2 == 0 else nc.scalar
                if eng is nc.vector:
                    eng.tensor_copy(
                        s_T2[:, :, row, half * HREST : (half + 1) * HREST],
                        p_t.transpose([0, 2, 1]),
                    )
                else:
                    eng.copy(
                        s_T2[:, :, row, half * HREST : (half + 1) * HREST],
                        p_t.transpose([0, 2, 1]),
                    )

            # --- T to DRAM (contiguous both sides) ---
            nc.sync.dma_start(td_d.ap(), s_T2)

            # --- 25 per-tap gathers into R[(t,h) 128p, tap, w] ---
            s_R = sb.tile([128, NTAP, W], f32)
            for k in range(NTAP):
                dyi, dxi = divmod(k, KS)
                src = bass.AP(
                    tensor=td_d.ap().tensor,
                    offset=k * NPOS + dyi * SLAB_W + dxi,
                    ap=[[SLAB_R * SLAB_W, S], [SLAB_W, ROWS], [1, W]],
                )
                nc.sync.dma_start(s_R[:, k, :], src)

            # --- tap reduce on 128 lanes (strided view puts tap innermost) ---
            s_sk = sb.tile([128, W], f32)  # [(t,h), w]
            nc.vector.tensor_reduce(
                s_sk, s_R.transpose([0, 2, 1]), axis=mybir.AxisListType.X, op=ALU.add
            )

            # --- bounce sK to [h, (t, w)] for the t-softmax ---
            nc.sync.dma_start(
                bass.AP(
                    tensor=skd_d.ap().tensor,
                    offset=0,
                    ap=[[ROWS * W, S], [W, ROWS], [1, W]],  # (t, h, w)
                ),
                s_sk,
            )
            s_sk8 = sb.tile([ROWS, S, W], f32)
            nc.sync.dma_start(
                s_sk8,
                bass.AP(
                    tensor=skd_d.ap().tensor,
                    offset=0,
                    ap=[[W, ROWS], [ROWS * W, S], [1, W]],  # (h, t, w)
                ),
            )

            # --- softmax over t (scores are small: skip max subtraction) ---
            s_e8 = sb.tile([ROWS, S, W], f32)
            nc.scalar.activation(s_e8, s_sk8, AF.Exp)
            s_den = sb.tile([ROWS, W], f32)
            nc.vector.tensor_reduce(
                s_den, s_e8.transpose([0, 2, 1]), axis=mybir.AxisListType.X, op=ALU.add
            )
            s_rcp = sb.tile([ROWS, W], f32)
            nc.vector.reciprocal(s_rcp, s_den)
            s_attn = sb.tile([ROWS, S, W], f32)
            nc.vector.tensor_tensor(
                s_attn,
                s_e8,
                s_rcp.unsqueeze(1).broadcast_to((ROWS, S, W)),
                op=ALU.mult,
            )

            # --- bounce attn to [(ci8,t), (h,w)] ---
            nc.sync.dma_start(
                bass.AP(
                    tensor=attnd_d.ap().tensor,
                    offset=0,
                    ap=[[W, ROWS], [ROWS * W, S], [1, W]],  # (h, t, w)
                ),
                s_attn,
            )
            s_attnb = sb.tile([128, HW], f32)
            for g in range(8):
                src = bass.AP(
                    tensor=attnd_d.ap().tensor,
                    offset=0,
                    ap=[[ROWS * W, S], [1, ROWS * W]],  # (t, hw)
                )
                nc.sync.dma_start(s_attnb[g * S : (g + 1) * S, :], src)

            # --- V path: xattn = x_t * attn; contract (ci,t) on PE ---
            s_xa = sb.tile([128, 8, HW], f32)
            nc.vector.tensor_tensor(
                s_xa,
                s_xt,
                s_attnb.unsqueeze(1).broadcast_to((128, 8, HW)),
                op=ALU.mult,
            )
            p_o = pso.tile([C, HW], f32)
            for g in range(8):
                nc.tensor.matmul(
                    p_o,
                    s_w1vr[:, g, :],
                    s_xa[:, g, :],
                    start=(g == 0),
                    stop=(g == 7),
                )
            s_o = sb.tile([C, HW], f32)
            nc.scalar.copy(s_o, p_o)
            nc.sync.dma_start(
                o_d.ap(), s_o.unsqueeze(1).broadcast_to((C, S, HW))
            )

    nc.compile()
    return nc


def _get_module():
    global _MODULE
    if _MODULE is None:
        _MODULE = _build_module()
    return _MODULE


def make_host_inputs(x, w1, b1, w2, b2):
    """Host-side precompute: folded weights + per-core reflect-padded slices."""
    x = np.ascontiguousarray(np.asarray(x, np.float32))
    w1 = np.asarray(w1, np.float32)
    w2 = np.asarray(w2, np.float32)

    w1K = w1[C : 2 * C, :, 0, 0]          # [c, ci]
    w2K = w2[0, C : 2 * C]                # [c, 5, 5]
    weff = np.ascontiguousarray(
        np.einsum("ci,cyx->iyx", w1K, w2K).reshape(C, NTAP)
    )
    w1V = w1[2 * C :, :, 0, 0]            # [co, ci]

    # w1vr[(ci8,t), g, co] = w1V[co, 8g+ci8]
    tmp = w1V.T.reshape(8, 8, C)                      # (g, ci8, co)
    w1vr = np.ascontiguousarray(
        np.broadcast_to(tmp[:, :, None, :], (8, 8, S, C))
        .transpose(1, 2, 0, 3)
        .reshape(128, 8, C)
    )

    in_maps = []
    for core in range(NCORES):
        b, hc = divmod(core, 4)
        h0 = ROWS * hc
        xp = np.pad(x[b], ((0, 0), (PAD, PAD), (PAD, PAD), (0, 0)), mode="reflect")
        slab = np.ascontiguousarray(xp[:, h0 : h0 + SLAB_R, :, :])
        xs = x[b][:, h0 : h0 + ROWS, :, :]            # [ci, h, w, t]
        xt = np.ascontiguousarray(
            xs.reshape(8, 8, ROWS, W, S)
            .transpose(1, 4, 0, 2, 3)
            .reshape(128, 8, HW)
        )
        in_maps.append({"slab": slab, "xt": xt, "weff": weff, "w1vr": w1vr})
    return in_maps


def assemble_output(results, b1):
    b1V = np.asarray(b1, np.float32)[2 * C :]
    out = np.empty((B, C, H, W, S), np.float32)
    for core in range(NCORES):
        b, hc = divmod(core, 4)
        h0 = ROWS * hc
        o = results[core]["o"].reshape(C, S, ROWS, W).transpose(0, 2, 3, 1)
        out[b, :, h0 : h0 + ROWS, :, :] = o
    out += b1V[None, :, None, None, None]
    return out


def kernel(x, w1, b1, w2, b2):
    from concourse.bass_utils import run_bass_kernel_spmd

    nc = _get_module()
    in_maps = make_host_inputs(x, w1, b1, w2, b2)
    res = run_bass_kernel_spmd(nc, in_maps, core_ids=list(range(NCORES)))
    return assemble_output(res.results, b1)


# revision 12
# speedup vs baseline: 1.8134x; 1.2322x over previous
"""Trainium2 Bass kernel for nn_ConvAttention_34600256537137.

Math notes (validated against the reference to ~3e-6 rel err):
  qkv = 1x1conv(x, w1)+b1 -> Q,K,V;  score = conv5x5(Q_s)+conv5x5(K_t)+b2;
  attn = softmax_t(score);  out = einsum(attn, V).
  Softmax over t is shift-invariant, so the Q-half of the score (constant in
  t), b2, and the K-path bias all cancel.  The computation collapses to:
    weff[ci,dy,dx] = sum_c w1K[c,ci] * w2K[c,dy,dx]        (host, tiny)
    sK[b,t,h,w]    = conv5x5_reflect(x[b,:,:,:,t], weff)
    e = exp(sK);  den = sum_t e
    out[b,o,h,w,s] = (sum_{ci,t} w1V[o,ci] * e * x) / den + b1V[o]
  (s-independent; normalization folded to the end; bias added on host)

Sharding: 8 cores = (b in {0,1}) x (4 chunks of 8 rows of H).  All reflect
padding and layout transforms are precomputed host-side so every core runs an
identical program on its own slices.

Perf structure (v3):
  - DMAs are spread round-robin over all 5 engine queues (a single queue
    serializes descriptors at ~600ns each).
  - slab arrives as 6 row-pair tiles so conv matmuls start as rows land.
  - PE warmup matmuls run during the load window to trigger HAM clock
    promotion (1.2 -> 2.4 GHz) before the real conv.
  - score conv: T[tap,pos] = weff^T @ slab streamed once (24 half-row
    matmuls); PSUM->SBUF copies write T as (tap, t, row, w); DRAM bounce +
    25 per-tap gathers build R[(t,h), tap, w]; DVE reduces taps on 128 lanes.
  - softmax denominator via indicator-matmul on PE (no partition reduce);
    normalization happens on the final PSUM->SBUF read.
"""

import sys

if "/opt/trn_rl_repo" not in sys.path:
    sys.path.insert(0, "/opt/trn_rl_repo")

import numpy as np

B, C, H, W, S = 2, 64, 32, 32, 16
KS, PAD = 5, 2
NCORES = 8
ROWS = H // 4            # output rows per core
SLAB_R = ROWS + 2 * PAD  # 12
SLAB_W = W + 2 * PAD     # 36
NTAP = KS * KS           # 25
NPOS = SLAB_R * SLAB_W * S  # 6912 slab positions
HW = ROWS * W            # 256 output positions
NWARM = 64               # PE warmup matmuls

_MODULE = None


def _build_module():
    import concourse.bacc as bacc
    import concourse.bass as bass
    import concourse.tile as tile
    from concourse import mybir

    f32 = mybir.dt.float32
    AF = mybir.ActivationFunctionType
    ALU = mybir.AluOpType
    nc = bacc.Bacc("TRN2", target_bir_lowering=False, debug=False, num_devices=NCORES)

    slab_d = nc.dram_tensor("slab", [C, SLAB_R, SLAB_W, S], f32, kind="ExternalInput")
    xt_d = nc.dram_tensor("xt", [128, 8, HW], f32, kind="ExternalInput")
    weff_d = nc.dram_tensor("weff", [C, NTAP], f32, kind="ExternalInput")
    w1vr_d = nc.dram_tensor("w1vr", [128, 8, C], f32, kind="ExternalInput")
    hsel_d = nc.dram_tensor("hsel", [128, ROWS], f32, kind="ExternalInput")
    o_d = nc.dram_tensor("o", [C, S, HW], f32, kind="ExternalOutput")

    # scratch DRAM for partition-crossing rearrangements
    td_d = nc.dram_tensor("td", [NTAP, S, SLAB_R, SLAB_W], f32)   # T, t-major
    ed_d = nc.dram_tensor("ed", [S, ROWS, W], f32)                # exp(sK), t-major
    dend_d = nc.dram_tensor("dend", [ROWS * W], f32)              # 1/den, flat hw

    engs = None
    _rr = [0]

    def dma(out, in_):
        e = engs[_rr[0] % len(engs)]
        _rr[0] += 1
        e.dma_start(out, in_)

    with tile.TileContext(nc) as tc:
        engs = [nc.sync, nc.scalar, nc.gpsimd]
        with tc.tile_pool(name="sb", bufs=1) as sb, tc.tile_pool(
            name="ps", bufs=4, space="PSUM"
        ) as ps, tc.tile_pool(name="psw", bufs=2, space="PSUM") as psw, tc.tile_pool(
            name="pso", bufs=1, space="PSUM"
        ) as pso:
            # --- loads: weff first (feeds warmups), slab as 6 row-pair tiles
            s_weff = sb.tile([C, NTAP], f32)
            nc.sync.dma_start(s_weff, weff_d.ap())
            s_hsel = sb.tile([128, ROWS], f32)
            nc.gpsimd.dma_start(s_hsel, hsel_d.ap())
            slab_t = []
            for rp in range(6):
                t = sb.tile([C, 2, SLAB_W, S], f32, tag=f"slab{rp}")
                dma(t, slab_d.ap()[:, 2 * rp : 2 * rp + 2, :, :])
                slab_t.append(t)
            s_xt = sb.tile([128, 8, HW], f32)
            nc.sync.dma_start(s_xt, xt_d.ap())
            s_w1vr = sb.tile([128, 8, C], f32)
            nc.scalar.dma_start(s_w1vr, w1vr_d.ap())

            # --- PE warmup during the load window (HAM clock promotion) ---
            for j in range(NWARM):
                p_w = psw.tile([NTAP, NTAP], f32, tag="warm")
                nc.tensor.matmul(p_w, s_weff, s_weff, start=True, stop=True)

            # --- phase 1: T[tap, (row, w, t)] = weff^T @ slab, half-row chunks
            # s_T2 holds T transposed to (tap, t, row, w): w contiguous.
            s_T2 = sb.tile([NTAP, S, SLAB_R, SLAB_W], f32)
            HREST = SLAB_W // 2  # 18
            for hr in range(SLAB_R * 2):
                row, half = divmod(hr, 2)
                p_t = ps.tile([NTAP, HREST, S], f32, tag="pt")
                nc.tensor.matmul(
                    p_t,
                    s_weff,
                    slab_t[row // 2][:, row % 2, half * HREST : (half + 1) * HREST, :],
                    start=True,
                    stop=True,
                )
                # copy PSUM -> s_T2[(tap), t, row, w-half] (strided write)
                eng = nc.vector if hr % 2 == 0 else nc.scalar
                if eng is nc.vector:
                    eng.tensor_copy(
                        s_T2[:, :, row, half * HREST : (half + 1) * HREST],
                        p_t.transpose([0, 2, 1]),
                    )
                else:
                    eng.copy(
                        s_T2[:, :, row, half * HREST : (half + 1) * HREST],
                        p_t.transpose([0, 2, 1]),
                    )

            # --- T to DRAM (contiguous both sides) ---
            nc.sync.dma_start(td_d.ap(), s_T2)

            # --- 25 per-tap gathers into R[(t,h) 128p, tap, w] ---
            s_R = sb.tile([128, NTAP, W], f32)
            for k in range(NTAP):
                dyi, dxi = divmod(k, KS)
                src = bass.AP(
                    tensor=td_d.ap().tensor,
                    offset=k * NPOS + dyi * SLAB_W + dxi,
                    ap=[[SLAB_R * SLAB_W, S], [SLAB_W, ROWS], [1, W]],
                )
                dma(s_R[:, k, :], src)

            # --- tap reduce on 128 lanes (strided view puts tap innermost) ---
            s_sk = sb.tile([128, W], f32)  # [(t,h), w]
            nc.vector.tensor_reduce(
                s_sk, s_R.transpose([0, 2, 1]), axis=mybir.AxisListType.X, op=ALU.add
            )

            # --- e = exp(sK) in [(t,h), w]; den via indicator-matmul on PE ---
            s_e = sb.tile([128, W], f32)
            nc.scalar.activation(s_e, s_sk, AF.Exp)
            p_den = pso.tile([ROWS, W], f32, tag="den")
            nc.tensor.matmul(p_den, s_hsel, s_e, start=True, stop=True)
            s_rcp = sb.tile([ROWS, W], f32)
            nc.vector.reciprocal(s_rcp, p_den)
            nc.gpsimd.dma_start(dend_d.ap(), s_rcp)
            s_rcpb = sb.tile([C, HW], f32)
            nc.gpsimd.dma_start(
                s_rcpb,
                bass.AP(tensor=dend_d.ap().tensor, offset=0, ap=[[0, C], [1, HW]]),
            )

            # --- bounce e to [t, hw] and read back as [(ci8,t), hw] ---
            # (s_e partitions iterate (t, h) so the flat [t, h, w] layout of
            # ed_d matches the source order directly)
            nc.sync.dma_start(ed_d.ap(), s_e)
            s_eb = sb.tile([128, HW], f32)
            for g in range(8):
                src = bass.AP(
                    tensor=ed_d.ap().tensor,
                    offset=0,
                    ap=[[ROWS * W, S], [1, ROWS * W]],  # (t, hw)
                )
                dma(s_eb[g * S : (g + 1) * S, :], src)

            # --- V path: xattn = x_t * e; contract (ci,t) on PE ---
            s_xa = sb.tile([128, 8, HW], f32)
            nc.vector.tensor_tensor(
                s_xa,
                s_xt,
                s_eb.unsqueeze(1).broadcast_to((128, 8, HW)),
                op=ALU.mult,
            )
            p_o = pso.tile([C, HW], f32, tag="out")
            for g in range(8):
                nc.tensor.matmul(
                    p_o,
                    s_w1vr[:, g, :],
                    s_xa[:, g, :],
                    start=(g == 0),
                    stop=(g == 7),
                )
            # normalize on the PSUM->SBUF read
            s_o = sb.tile([C, HW], f32)
            nc.vector.tensor_tensor(s_o, p_o, s_rcpb, op=ALU.mult)
            nc.sync.dma_start(
                o_d.ap(), s_o.unsqueeze(1).broadcast_to((C, S, HW))
            )

    nc.compile()
    return nc


def _get_module():
    global _MODULE
    if _MODULE is None:
        _MODULE = _build_module()
    return _MODULE


def make_host_inputs(x, w1, b1, w2, b2):
    """Host-side precompute: folded weights + per-core reflect-padded slices."""
    x = np.ascontiguousarray(np.asarray(x, np.float32))
    w1 = np.asarray(w1, np.float32)
    w2 = np.asarray(w2, np.float32)

    w1K = w1[C : 2 * C, :, 0, 0]          # [c, ci]
    w2K = w2[0, C : 2 * C]                # [c, 5, 5]
    weff = np.ascontiguousarray(
        np.einsum("ci,cyx->iyx", w1K, w2K).reshape(C, NTAP)
    )
    w1V = w1[2 * C :, :, 0, 0]            # [co, ci]

    # w1vr[(ci8,t), g, co] = w1V[co, 8g+ci8]
    tmp = w1V.T.reshape(8, 8, C)                      # (g, ci8, co)
    w1vr = np.ascontiguousarray(
        np.broadcast_to(tmp[:, :, None, :], (8, 8, S, C))
        .transpose(1, 2, 0, 3)
        .reshape(128, 8, C)
    )

    # hsel[(t,h), m] = 1 if h == m  (partition index = t*ROWS + h)
    hsel = np.zeros((128, ROWS), np.float32)
    for t in range(S):
        for h in range(ROWS):
            hsel[t * ROWS + h, h] = 1.0

    in_maps = []
    for core in range(NCORES):
        b, hc = divmod(core, 4)
        h0 = ROWS * hc
        xp = np.pad(x[b], ((0, 0), (PAD, PAD), (PAD, PAD), (0, 0)), mode="reflect")
        slab = np.ascontiguousarray(xp[:, h0 : h0 + SLAB_R, :, :])
        xs = x[b][:, h0 : h0 + ROWS, :, :]            # [ci, h, w, t]
        xt = np.ascontiguousarray(
            xs.reshape(8, 8, ROWS, W, S)
            .transpose(1, 4, 0, 2, 3)
            .reshape(128, 8, HW)
        )
        in_maps.append(
            {"slab": slab, "xt": xt, "weff": weff, "w1vr": w1vr, "hsel": hsel}
        )
    return in_maps


def assemble_output(results, b1):
    b1V = np.asarray(b1, np.float32)[2 * C :]
    out = np.empty((B, C, H, W, S), np.float32)
    for core in range(NCORES):
        b, hc = divmod(core, 4)
        h0 = ROWS * hc
        o = results[core]["o"].reshape(C, S, ROWS, W).transpose(0, 2, 3, 1)
        out[b, :, h0 : h0 + ROWS, :, :] = o
    out += b1V[None, :, None, None, None]
    return out


def kernel(x, w1, b1, w2, b2):
    from concourse.bass_utils import run_bass_kernel_spmd

    nc = _get_module()
    in_maps = make_host_inputs(x, w1, b1, w2, b2)
    res = run_bass_kernel_spmd(nc, in_maps, core_ids=list(range(NCORES)))
    return assemble_output(res.results, b1)


# revision 13
# speedup vs baseline: 1.9810x; 1.0924x over previous
"""Trainium2 Bass kernel for nn_ConvAttention_34600256537137.

Math notes (validated against the reference to ~3e-6 rel err):
  qkv = 1x1conv(x, w1)+b1 -> Q,K,V;  score = conv5x5(Q_s)+conv5x5(K_t)+b2;
  attn = softmax_t(score);  out = einsum(attn, V).
  Softmax over t is shift-invariant, so the Q-half of the score (constant in
  t), b2, and the K-path bias all cancel.  The computation collapses to:
    weff[ci,dy,dx] = sum_c w1K[c,ci] * w2K[c,dy,dx]        (host, tiny)
    sK[b,t,h,w]    = conv5x5_reflect(x[b,:,:,:,t], weff)
    e = exp(sK);  den = sum_t e
    out[b,o,h,w,s] = (sum_{ci,t} w1V[o,ci] * e * x) / den + b1V[o]
  (s-independent; normalization folded to the end; bias added on host)

Sharding: 8 cores = (b in {0,1}) x (4 chunks of 8 rows of H).  All reflect
padding and layout transforms are precomputed host-side so every core runs an
identical program on its own slices.

Perf structure (v3):
  - DMAs are spread round-robin over all 5 engine queues (a single queue
    serializes descriptors at ~600ns each).
  - slab arrives as 6 row-pair tiles so conv matmuls start as rows land.
  - score conv: T[tap,pos] = weff^T @ slab streamed once (24 half-row
    matmuls); PSUM->SBUF copies write T as (tap, t, row, w); DRAM bounce +
    25 per-tap gathers build R[(t,h), tap, w]; DVE reduces taps on 128 lanes.
  - softmax denominator via indicator-matmul on PE (no partition reduce);
    normalization happens on the final PSUM->SBUF read.
"""

import sys

if "/opt/trn_rl_repo" not in sys.path:
    sys.path.insert(0, "/opt/trn_rl_repo")

import numpy as np

B, C, H, W, S = 2, 64, 32, 32, 16
KS, PAD = 5, 2
NCORES = 8
ROWS = H // 4            # output rows per core
SLAB_R = ROWS + 2 * PAD  # 12
SLAB_W = W + 2 * PAD     # 36
NTAP = KS * KS           # 25
NPOS = SLAB_R * SLAB_W * S  # 6912 slab positions
HW = ROWS * W            # 256 output positions

_MODULE = None


def _build_module():
    import concourse.bacc as bacc
    import concourse.bass as bass
    import concourse.tile as tile
    from concourse import mybir

    f32 = mybir.dt.float32
    AF = mybir.ActivationFunctionType
    ALU = mybir.AluOpType
    nc = bacc.Bacc("TRN2", target_bir_lowering=False, debug=False, num_devices=NCORES)

    slab_d = nc.dram_tensor("slab", [C, SLAB_R, SLAB_W, S], f32, kind="ExternalInput")
    xt_d = nc.dram_tensor("xt", [128, 8, HW], f32, kind="ExternalInput")
    weff_d = nc.dram_tensor("weff", [C, NTAP], f32, kind="ExternalInput")
    w1vr_d = nc.dram_tensor("w1vr", [128, 8, C], f32, kind="ExternalInput")
    hsel_d = nc.dram_tensor("hsel", [128, ROWS], f32, kind="ExternalInput")
    o_d = nc.dram_tensor("o", [C, S, HW], f32, kind="ExternalOutput")

    # scratch DRAM for partition-crossing rearrangements
    td_d = nc.dram_tensor("td", [NTAP, S, SLAB_R, SLAB_W], f32)   # T, t-major
    ed_d = nc.dram_tensor("ed", [S, ROWS, W], f32)                # exp(sK), t-major
    dend_d = nc.dram_tensor("dend", [ROWS * W], f32)              # 1/den, flat hw

    engs = None
    _rr = [0]

    def dma(out, in_):
        e = engs[_rr[0] % len(engs)]
        _rr[0] += 1
        e.dma_start(out, in_)

    with tile.TileContext(nc) as tc:
        engs = [nc.sync, nc.scalar, nc.gpsimd]
        with tc.tile_pool(name="sb", bufs=1) as sb, tc.tile_pool(
            name="ps", bufs=6, space="PSUM"
        ) as ps, tc.tile_pool(name="pso", bufs=1, space="PSUM") as pso:
            # --- loads: weff first (feeds warmups), slab as 6 row-pair tiles
            s_weff = sb.tile([C, NTAP], f32)
            nc.sync.dma_start(s_weff, weff_d.ap())
            s_hsel = sb.tile([128, ROWS], f32)
            nc.gpsimd.dma_start(s_hsel, hsel_d.ap())
            slab_t = []
            for rp in range(6):
                t = sb.tile([C, 2, SLAB_W, S], f32, tag=f"slab{rp}")
                dma(t, slab_d.ap()[:, 2 * rp : 2 * rp + 2, :, :])
                slab_t.append(t)
            s_xt = sb.tile([128, 8, HW], f32)
            nc.sync.dma_start(s_xt, xt_d.ap())
            s_w1vr = sb.tile([128, 8, C], f32)
            nc.scalar.dma_start(s_w1vr, w1vr_d.ap())

            # --- phase 1: T[tap, (row, w, t)] = weff^T @ slab, half-row chunks
            # s_T2 holds T transposed to (tap, t, row, w): w contiguous.
            s_T2 = sb.tile([NTAP, S, SLAB_R, SLAB_W], f32)
            HREST = SLAB_W // 2  # 18
            for hr in range(SLAB_R * 2):
                row, half = divmod(hr, 2)
                p_t = ps.tile([NTAP, HREST, S], f32, tag="pt")
                nc.tensor.matmul(
                    p_t,
                    s_weff,
                    slab_t[row // 2][:, row % 2, half * HREST : (half + 1) * HREST, :],
                    start=True,
                    stop=True,
                )
                # copy PSUM -> s_T2[(tap), t, row, w-half] (strided write)
                eng = nc.vector if hr % 2 == 0 else nc.scalar
                if eng is nc.vector:
                    eng.tensor_copy(
                        s_T2[:, :, row, half * HREST : (half + 1) * HREST],
                        p_t.transpose([0, 2, 1]),
                    )
                else:
                    eng.copy(
                        s_T2[:, :, row, half * HREST : (half + 1) * HREST],
                        p_t.transpose([0, 2, 1]),
                    )

            # --- T to DRAM (contiguous both sides), 3 row-chunks in parallel
            for ci, e in enumerate((nc.sync, nc.scalar, nc.gpsimd)):
                e.dma_start(
                    td_d.ap()[:, :, 4 * ci : 4 * ci + 4, :],
                    s_T2[:, :, 4 * ci : 4 * ci + 4, :],
                )

            # --- 25 per-tap gathers into R[(t,h) 128p, tap, w] ---
            s_R = sb.tile([128, NTAP, W], f32)
            for k in range(NTAP):
                dyi, dxi = divmod(k, KS)
                src = bass.AP(
                    tensor=td_d.ap().tensor,
                    offset=k * NPOS + dyi * SLAB_W + dxi,
                    ap=[[SLAB_R * SLAB_W, S], [SLAB_W, ROWS], [1, W]],
                )
                dma(s_R[:, k, :], src)

            # --- tap reduce on 128 lanes (strided view puts tap innermost) ---
            s_sk = sb.tile([128, W], f32)  # [(t,h), w]
            nc.vector.tensor_reduce(
                s_sk, s_R.transpose([0, 2, 1]), axis=mybir.AxisListType.X, op=ALU.add
            )

            # --- e = exp(sK) in [(t,h), w]; den via indicator-matmul on PE ---
            s_e = sb.tile([128, W], f32)
            nc.scalar.activation(s_e, s_sk, AF.Exp)
            p_den = pso.tile([ROWS, W], f32, tag="den")
            nc.tensor.matmul(p_den, s_hsel, s_e, start=True, stop=True)
            s_rcp = sb.tile([ROWS, W], f32)
            nc.vector.reciprocal(s_rcp, p_den)
            nc.gpsimd.dma_start(dend_d.ap(), s_rcp)
            s_rcpb = sb.tile([C, HW], f32)
            nc.gpsimd.dma_start(
                s_rcpb,
                bass.AP(tensor=dend_d.ap().tensor, offset=0, ap=[[0, C], [1, HW]]),
            )

            # --- bounce e to [t, hw] and read back as [(ci8,t), hw] ---
            # (s_e partitions iterate (t, h) so the flat [t, h, w] layout of
            # ed_d matches the source order directly)
            nc.sync.dma_start(ed_d.ap(), s_e)
            s_eb = sb.tile([128, HW], f32)
            for g in range(8):
                src = bass.AP(
                    tensor=ed_d.ap().tensor,
                    offset=0,
                    ap=[[ROWS * W, S], [1, ROWS * W]],  # (t, hw)
                )
                dma(s_eb[g * S : (g + 1) * S, :], src)

            # --- V path: xattn = x_t * e; contract (ci,t) on PE ---
            s_xa = sb.tile([128, 8, HW], f32)
            nc.vector.tensor_tensor(
                s_xa,
                s_xt,
                s_eb.unsqueeze(1).broadcast_to((128, 8, HW)),
                op=ALU.mult,
            )
            p_o = pso.tile([C, HW], f32, tag="out")
            for g in range(8):
                nc.tensor.matmul(
                    p_o,
                    s_w1vr[:, g, :],
                    s_xa[:, g, :],
                    start=(g == 0),
                    stop=(g == 7),
                )
            # normalize on the PSUM->SBUF read
            s_o = sb.tile([C, HW], f32)
            nc.vector.tensor_tensor(s_o, p_o, s_rcpb, op=ALU.mult)
            for ci, e in enumerate((nc.sync, nc.scalar)):
                half = C // 2
                e.dma_start(
                    o_d.ap()[ci * half : (ci + 1) * half],
                    s_o[ci * half : (ci + 1) * half].unsqueeze(1).broadcast_to(
                        (half, S, HW)
                    ),
                )

    nc.compile()
    return nc


def _get_module():
    global _MODULE
    if _MODULE is None:
        _MODULE = _build_module()
    return _MODULE


def make_host_inputs(x, w1, b1, w2, b2):
    """Host-side precompute: folded weights + per-core reflect-padded slices."""
    x = np.ascontiguousarray(np.asarray(x, np.float32))
    w1 = np.asarray(w1, np.float32)
    w2 = np.asarray(w2, np.float32)

    w1K = w1[C : 2 * C, :, 0, 0]          # [c, ci]
    w2K = w2[0, C : 2 * C]                # [c, 5, 5]
    weff = np.ascontiguousarray(
        np.einsum("ci,cyx->iyx", w1K, w2K).reshape(C, NTAP)
    )
    w1V = w1[2 * C :, :, 0, 0]            # [co, ci]

    # w1vr[(ci8,t), g, co] = w1V[co, 8g+ci8]
    tmp = w1V.T.reshape(8, 8, C)                      # (g, ci8, co)
    w1vr = np.ascontiguousarray(
        np.broadcast_to(tmp[:, :, None, :], (8, 8, S, C))
        .transpose(1, 2, 0, 3)
        .reshape(128, 8, C)
    )

    # hsel[(t,h), m] = 1 if h == m  (partition index = t*ROWS + h)
    hsel = np.zeros((128, ROWS), np.float32)
    for t in range(S):
        for h in range(ROWS):
            hsel[t * ROWS + h, h] = 1.0

    in_maps = []
    for core in range(NCORES):
        b, hc = divmod(core, 4)
        h0 = ROWS * hc
        xp = np.pad(x[b], ((0, 0), (PAD, PAD), (PAD, PAD), (0, 0)), mode="reflect")
        slab = np.ascontiguousarray(xp[:, h0 : h0 + SLAB_R, :, :])
        xs = x[b][:, h0 : h0 + ROWS, :, :]            # [ci, h, w, t]
        xt = np.ascontiguousarray(
            xs.reshape(8, 8, ROWS, W, S)
            .transpose(1, 4, 0, 2, 3)
            .reshape(128, 8, HW)
        )
        in_maps.append(
            {"slab": slab, "xt": xt, "weff": weff, "w1vr": w1vr, "hsel": hsel}
        )
    return in_maps


def assemble_output(results, b1):
    b1V = np.asarray(b1, np.float32)[2 * C :]
    out = np.empty((B, C, H, W, S), np.float32)
    for core in range(NCORES):
        b, hc = divmod(core, 4)
        h0 = ROWS * hc
        o = results[core]["o"].reshape(C, S, ROWS, W).transpose(0, 2, 3, 1)
        out[b, :, h0 : h0 + ROWS, :, :] = o
    out += b1V[None, :, None, None, None]
    return out


def kernel(x, w1, b1, w2, b2):
    from concourse.bass_utils import run_bass_kernel_spmd

    nc = _get_module()
    in_maps = make_host_inputs(x, w1, b1, w2, b2)
    res = run_bass_kernel_spmd(nc, in_maps, core_ids=list(range(NCORES)))
    return assemble_output(res.results, b1)
